# revision 15
# baseline (speedup 1.0000x reference)
"""AssimilationLoss Trainium2 kernel.

Reference math (x: [B, N, D] f32):
    loss = mean_b || sum_i x[b,i,:] / max(||x[b,i,:]||, eps) ||^2 / N^2

Sharding: data-parallel over B across 8 NeuronCores (one batch element per
core).  Each core streams its [N, D] shard once from HBM (16 MiB -> memory
bound), computes partial_b = || sum_i x_i/||x_i|| ||^2 locally, and the host
averages the 8 scalars.

Per-core pipeline over [128, 512] row-tiles:
  ACT : square + row-accumulate           -> ss[p]  = sum_d x[p,d]^2
  DVE : reciprocal (batched)              -> 1/ss
  ACT : sqrt (batched)                    -> inv[p] = 1/||x_p||
  DVE : tensor_scalar mul (fp32 2x mode)  -> xn = x * inv
  PE  : ones^T @ xn (float32r, PSUM acc)  -> s[1, D] += sum_p xn[p, :]
Epilogue: ACT square+acc of s -> scalar, DMA out.
"""

import numpy as np

import concourse.bacc as bacc
import concourse.mybir as mybir
import concourse.tile as tile
from concourse.bass_utils import run_bass_kernel_spmd


def _ensure_ntff_hook():
    """Provide antenv.axon_hooks (NTFF profiling glue) if the image lacks it."""
    try:
        from antenv.axon_hooks import get_axon_ntff_profile_hook  # noqa: F401

        return
    except ImportError:
        pass
    import contextlib
    import ctypes
    import sys
    import types

    so_path = "/opt/axon/libaxon_pjrt.so"
    mod = types.ModuleType("antenv.axon_hooks")
    _state = {"hook": None}
    mod.set_axon_ntff_profile_hook = lambda h: _state.__setitem__("hook", h)
    mod.get_axon_ntff_profile_hook = lambda: _state["hook"]
    try:
        lib = ctypes.CDLL(so_path)
        if hasattr(lib, "axon_start_nrt_profile"):
            lib.axon_start_nrt_profile.argtypes = [
                ctypes.POINTER(ctypes.c_int64),
                ctypes.c_size_t,
            ]
            lib.axon_start_nrt_profile.restype = ctypes.c_int64
            lib.axon_stop_nrt_profile.argtypes = [ctypes.c_char_p]
            lib.axon_stop_nrt_profile.restype = ctypes.c_int64

            @contextlib.contextmanager
            def _hook(output_dir, device_ids):
                import jax

                jax.devices()
                if device_ids:
                    ids = (ctypes.c_int64 * len(device_ids))(*device_ids)
                    rc = lib.axon_start_nrt_profile(ids, len(device_ids))
                else:
                    rc = lib.axon_start_nrt_profile(None, 0)
                if rc != 0:
                    raise RuntimeError(f"axon_start_nrt_profile rc={rc}")
                try:
                    yield
                finally:
                    n = lib.axon_stop_nrt_profile(str(output_dir).encode())
                    if n <= 0:
                        print(f"ntff profile: rc={n} (no files?)", file=sys.stderr)

            _state["hook"] = _hook
    except OSError:
        pass
    import antenv

    sys.modules["antenv.axon_hooks"] = mod
    antenv.axon_hooks = mod


_ensure_ntff_hook()

B, N, D = 8, 8192, 512
P = 128                      # SBUF partitions
ROWS_PER_CHUNK = 512         # rows DMA'd per transfer (1 MiB read)
N_SUB = ROWS_PER_CHUNK // P  # row-tiles per chunk
N_CHUNKS = N // ROWS_PER_CHUNK

F32 = mybir.dt.float32
F32R = mybir.dt.float32r
BF16 = mybir.dt.bfloat16

# row-tiles (of N_SUB per chunk) whose square+rowsum runs on ACT; rest on DVE
ACT_SUBTILES = {3}


USE_RAW = True


def _build_nc():
    nc = bacc.Bacc("TRN2", target_bir_lowering=False, debug=False)
    x_ext = nc.dram_tensor("x", [N, D], F32, kind="ExternalInput")
    out_ext = nc.dram_tensor("out", [1, 1], F32, kind="ExternalOutput")

    if USE_RAW:
        _body_raw(nc, x_ext.ap(), out_ext.ap())
    else:
        with tile.TileContext(nc) as tc:
            _body(tc, nc, x_ext.ap(), out_ext.ap())

    nc.compile()
    return nc


def _act_set(c):
    """Subtile indices (within a chunk) whose square+rowsum runs on ACT."""
    return ACT_SUBTILES if c % 2 else ACT_SUBTILES | {1}


def _body_raw(nc, x, out):
    """Raw Bacc version: manual semaphores, no Tile prologue/epilogue."""
    NT = N_CHUNKS * N_SUB  # total row-tiles (64)

    xt = nc.alloc_sbuf_tensor("xt", [P, N_CHUNKS, N_SUB, D], BF16).ap()
    ss = nc.alloc_sbuf_tensor("ss", [P, N_CHUNKS, N_SUB], F32).ap()
    rcp = nc.alloc_sbuf_tensor("rcp", [P, N_CHUNKS, N_SUB], F32).ap()
    inv = nc.alloc_sbuf_tensor("inv", [P, N_CHUNKS, N_SUB], BF16).ap()
    sq_a = nc.alloc_sbuf_tensor("sq_a", [P, D], BF16).ap()
    sq_v = nc.alloc_sbuf_tensor("sq_v", [P, D], BF16).ap()
    s_sq = nc.alloc_sbuf_tensor("s_sq", [1, D], F32).ap()
    partial = nc.alloc_sbuf_tensor("partial", [1, 1], F32).ap()

    x_chunks = x.rearrange("(c p n) d -> c p n d", p=P, n=N_SUB)

    with (
        nc.psum_tensor([1, D], F32) as s_acc,
        nc.semaphore("dma_sem") as dma_sem,
        nc.semaphore("ss_sem") as ss_sem,
        nc.semaphore("rcp_sem") as rcp_sem,
        nc.semaphore("inv_sem") as inv_sem,
        nc.semaphore("mm_sem") as mm_sem,
        nc.semaphore("fin_sem") as fin_sem,
        nc.Block() as block,
    ):

        @block.gpsimd
        def _(gpsimd):
            for c in range(N_CHUNKS):
                gpsimd.dma_start(
                    out=xt[:, c, :, :], in_=x_chunks[c]
                ).then_inc(dma_sem, 16)

        @block.scalar
        def _(scalar):
            # software-pipelined: squares of chunk c+1 before sqrt of chunk c
            def squares(c):
                scalar.wait_ge(dma_sem, 16 * (c + 1))
                for n in sorted(_act_set(c)):
                    scalar.activation(
                        out=sq_a,
                        in_=xt[:, c, n, :],
                        func=mybir.ActivationFunctionType.Square,
                        accum_out=ss[:, c, n : n + 1],
                    ).then_inc(ss_sem, 1)

            def sqrt(c):
                scalar.wait_ge(rcp_sem, c + 1)
                scalar.activation(
                    out=inv[:, c, :],
                    in_=rcp[:, c, :],
                    func=mybir.ActivationFunctionType.Sqrt,
                ).then_inc(inv_sem, 1)

            squares(0)
            for c in range(1, N_CHUNKS):
                squares(c)
                sqrt(c - 1)
            sqrt(N_CHUNKS - 1)

            # epilogue: partial = sum_d s[d]^2
            scalar.wait_ge(mm_sem, NT)
            scalar.activation(
                out=s_sq,
                in_=s_acc.ap(),
                func=mybir.ActivationFunctionType.Square,
                accum_out=partial,
            ).then_inc(fin_sem, 1)

        @block.vector
        def _(vector):
            n_act_done = 0
            for c in range(N_CHUNKS):
                vector.wait_ge(dma_sem, 16 * (c + 1))
                for n in range(N_SUB):
                    if n not in _act_set(c):
                        vector.affine_mul_reduce(
                            out=sq_v,
                            accum_out=ss[:, c, n : n + 1],
                            in0=xt[:, c, n, :],
                            in1=xt[:, c, n, :],
                            scale=1.0,
                            bias=0.0,
                        )
                n_act_done += len(_act_set(c))
                vector.wait_ge(ss_sem, n_act_done)
                vector.reciprocal(out=rcp[:, c, :], in_=ss[:, c, :]).then_inc(
                    rcp_sem, 1
                )

        @block.tensor
        def _(tensor):
            mm = 0
            for c in range(N_CHUNKS):
                tensor.wait_ge(inv_sem, c + 1)
                for n in range(N_SUB):
                    tensor.matmul(
                        s_acc.ap(),
                        inv[:, c, n : n + 1],
                        xt[:, c, n, :],
                        start=(mm == 0),
                        stop=(mm == NT - 1),
                    ).then_inc(mm_sem, 1)
                    mm += 1

        @block.sync
        def _(sync):
            sync.wait_ge(fin_sem, 1)
            sync.dma_start(out=out, in_=partial).then_inc(dma_sem, 16)
            sync.wait_ge(dma_sem, 16 * N_CHUNKS + 16)


def _body(tc, nc, x, out):
    import contextlib

    ctx = contextlib.ExitStack()
    with ctx:
        data = ctx.enter_context(tc.tile_pool(name="data", bufs=N_CHUNKS))
        small = ctx.enter_context(tc.tile_pool(name="small", bufs=4))
        sq = ctx.enter_context(tc.tile_pool(name="sq", bufs=2))
        sqd = ctx.enter_context(tc.tile_pool(name="sqd", bufs=2))
        singles = ctx.enter_context(tc.tile_pool(name="singles", bufs=1))
        psum = ctx.enter_context(tc.tile_pool(name="psum", bufs=1, space="PSUM"))

        s_acc = psum.tile([1, D], F32)

        # x viewed as chunks: rows c*RPC + 8*p + n  on partition p, slot n
        # -> per-partition contiguous 16 KiB DMA descriptors.
        x_chunks = x.rearrange("(c p n) d -> c p n d", p=P, n=N_SUB)

        mm = 0
        for c in range(N_CHUNKS):
            # SWDGE cast f32 -> bf16 during the DMA.
            xt = data.tile([P, N_SUB, D], BF16)
            nc.gpsimd.dma_start(out=xt, in_=x_chunks[c])

            # ss[p, n] = sum_d x[p,n,d]^2  (fp32 accum).  Split the row-tiles
            # between ACT (activation Square + accum) and DVE (tensor_tensor_
            # reduce, 2x bf16 mode) so neither engine is the bottleneck.
            act_subtiles = ACT_SUBTILES if c % 2 else ACT_SUBTILES | {1}
            ss = small.tile([P, N_SUB], F32, tag="ss")
            for n in range(N_SUB):
                if n in act_subtiles:
                    sq_t = sq.tile([P, D], BF16)
                    nc.scalar.activation(
                        out=sq_t,
                        in_=xt[:, n, :],
                        func=mybir.ActivationFunctionType.Square,
                        accum_out=ss[:, n : n + 1],
                    )
                else:
                    sq_d = sqd.tile([P, D], BF16)
                    nc.vector.affine_mul_reduce(
                        out=sq_d,
                        accum_out=ss[:, n : n + 1],
                        in0=xt[:, n, :],
                        in1=xt[:, n, :],
                        scale=1.0,
                        bias=0.0,
                    )

            rcp = small.tile([P, N_SUB], F32, tag="rcp")
            nc.vector.reciprocal(out=rcp, in_=ss)
            inv = small.tile([P, N_SUB], BF16, tag="inv")
            nc.scalar.activation(
                out=inv, in_=rcp, func=mybir.ActivationFunctionType.Sqrt
            )

            # s += inv_tile^T @ x_tile  : the per-row 1/||x|| scaling is
            # folded into the matmul weights; PSUM accumulates s[1, D].
            for n in range(N_SUB):
                nc.tensor.matmul(
                    s_acc,
                    inv[:, n : n + 1],
                    xt[:, n, :],
                    start=(mm == 0),
                    stop=(mm == N_CHUNKS * N_SUB - 1),
                )
                mm += 1

        # partial = sum_d s[d]^2
        s_sq = singles.tile([1, D], F32)
        partial = singles.tile([1, 1], F32)
        nc.scalar.activation(
            out=s_sq,
            in_=s_acc,
            func=mybir.ActivationFunctionType.Square,
            accum_out=partial,
        )
        nc.sync.dma_start(out=out, in_=partial)


_NC_CACHE = {}


def _get_nc():
    if "nc" not in _NC_CACHE:
        _NC_CACHE["nc"] = _build_nc()
    return _NC_CACHE["nc"]


def kernel(x: np.ndarray, trace: bool = False):
    assert x.shape == (B, N, D), x.shape
    nc = _get_nc()
    in_maps = [{"x": np.ascontiguousarray(x[b], dtype=np.float32)} for b in range(B)]
    res = run_bass_kernel_spmd(nc, in_maps, core_ids=list(range(B)), trace=trace)
    partials = [float(r["out"][0, 0]) for r in res.results]
    val = np.float32(np.sum(np.asarray(partials, dtype=np.float64)) / (N * N) / B)
    if trace:
        return val, res
    return val
